# revision 20
# baseline (speedup 1.0000x reference)
"""Trainium2 Bass kernel for nn_AttentionLayer (B=8, H=W=64, C=256, D=128).

Strategy: data-parallel over batch B=8 across the 8 NeuronCores (attention is
independent per batch element). Per core, for its batch element's x [L=4096,
C=256]:

  phase 1: PE-transpose x tiles (f32r, 1.5 c/r) -> xT, project q^T,k^T [D, L]
           (f32r) and v^T, then PE-transpose v^T -> v [L, D] (bf16).
           PSUM->SBUF copies split between ACT and DVE (both idle here).
  phase 2 (per 512-wide Lq chunk, per pair of 128-row Lk tiles,
           software-pipelined):
        MM1 pair: S^T [128,2,512] = k_tile @ q_chunk^T     (PE, fp32r,
                  into a 2-bank PSUM pair tile)
        exp:      ONE activation instr per pair -> bf16 P~^T into an oct
                  buffer [128,4,2,512]                      (ACT)
        MM2:      A~^T += v_tile^T @ P~^T tile              (PE, bf16 moving)
        denominator: bf16 DVE reduction tree, oct-batched (1285/751/485 ns
                  strided tensor_tensor adds), full depth -> ONE
                  MM3 (ones^T @ treetop) per chunk          (PE)
      tail (split in two, deferred into the next chunk's matmul stream):
        denom -> per-partition scale via 4 tiny PE transposes into the MM4
        PSUM bank, reciprocal (gamma folded into Wlast at setup);
        MM4: out = A~ @ Wlast (2+2 matmuls, single shared PSUM bank);
        DVE: out*scale + x; one batched output DMA per chunk.

Engine balance (cost-model): ACT ~133us (128 paired exps - the bottleneck),
PE ~132us, DVE ~105us (tree+tails), Pool ~10us. PSUM: 2x2-bank S pairs +
2 acc + 1 den + 1 MM4/scale = 8 banks.

Numerics: matmuls fp32r (MM1/projections) and bf16 (MM2/MM3 moving side)
with fp32 PSUM accumulation. Softmax skips max-subtraction: logits are
O(+-45) so exp stays inside fp32/bf16 range and softmax is shift-invariant.
P~, v, and the denominator tree are bf16 (~0.2% relative), attention
weights' bf16 error largely cancels after normalization.
"""

import numpy as np

import concourse.bass as bass
import concourse.mybir as mybir
import concourse.tile as tile
from concourse import bacc
from concourse.masks import make_identity
from concourse.bass_utils import run_bass_kernel_spmd

f32 = mybir.dt.float32
f32r = mybir.dt.float32r
bf16 = mybir.dt.bfloat16
AF = mybir.ActivationFunctionType
ALU = mybir.AluOpType

B, H, W, C, D = 8, 64, 64, 256, 128
L = H * W            # 4096
NT = L // 128        # 32 L-tiles of 128 rows
NCHUNK = L // 512    # 8 Lq chunks of 512
CK = C // 128        # 2 C-chunks
NG = NT // 2         # 16 tile-pairs (groups) per chunk
PIPE_DEPTH = 3       # groups in flight before MM2 drains


def _emit(nc, tc, ctx, nreps=1):
    x_d = nc.declare_dram_parameter("x", [L, C], f32, isOutput=False)
    wq_d = nc.declare_dram_parameter("Wq", [C, D], f32, isOutput=False)
    wk_d = nc.declare_dram_parameter("Wk", [C, D], f32, isOutput=False)
    wv_d = nc.declare_dram_parameter("Wv", [C, D], f32, isOutput=False)
    wl_d = nc.declare_dram_parameter("Wlast", [D, C], f32, isOutput=False)
    g_d = nc.declare_dram_parameter("gamma", [1], f32, isOutput=False)
    out_d = nc.declare_dram_parameter("out", [L, C], f32, isOutput=True)

    x_tiled = x_d[:].rearrange("(t p) c -> p t c", p=128)      # [128, NT, C]
    out_tiled = out_d[:].rearrange("(t p) c -> p t c", p=128)  # [128, NT, C]

    const = ctx.enter_context(tc.tile_pool(name="const", bufs=1))
    resident = ctx.enter_context(tc.tile_pool(name="resident", bufs=1))

    # --- constants (weight DMAs first: tiny, needed by the projections) --
    gamma_sb = const.tile([128, 1], f32)
    nc.sync.dma_start(out=gamma_sb[:], in_=g_d[:].to_broadcast((128, 1)))
    w_r = {}
    wtmps = []
    for name, wd in (("q", wq_d), ("k", wk_d), ("v", wv_d)):
        wtmp = const.tile([128, CK, D], f32, name=f"wtmp_{name}")
        nc.sync.dma_start(out=wtmp[:], in_=wd[:].rearrange("(cc p) d -> p cc d", p=128))
        wtmps.append((name, wtmp))
    wl_tmp = const.tile([128, C], f32)
    nc.sync.dma_start(out=wl_tmp[:], in_=wl_d[:])

    # x DMAs right behind (small first slice) so the PE can start
    # transposing ~3us in
    x_sb = resident.tile([128, NT, C], f32, tag="x_sb")      # 32 KB/part
    nc.sync.dma_start(out=x_sb[:, 0:4, :], in_=x_tiled[:, 0:4, :])
    for s in range(3):
        nc.sync.dma_start(
            out=x_sb[:, 4 + s * 8:12 + s * 8, :],
            in_=x_tiled[:, 4 + s * 8:12 + s * 8, :],
        )
    nc.sync.dma_start(out=x_sb[:, 28:32, :], in_=x_tiled[:, 28:32, :])

    identity = const.tile([128, 128], f32)
    make_identity(nc, identity[:])
    identity_r = const.tile([128, 128], f32r)
    nc.vector.tensor_copy(out=identity_r[:], in_=identity[:])
    ones_b = const.tile([128, 1], bf16)
    nc.vector.memset(ones_b[:], 1.0)
    id1 = const.tile([1, 1], f32)
    nc.vector.memset(id1[:], 1.0)
    # weights: lhsT chunks [C128, D] for q/k/v (f32r); [D, C] for last with
    # gamma folded in (f32r)
    for name, wtmp in wtmps:
        wr = const.tile([128, CK, D], f32r, name=f"w_{name}")
        nc.vector.tensor_copy(out=wr[:], in_=wtmp[:])
        w_r[name] = wr
    wl_r = const.tile([128, C], f32r)
    nc.vector.tensor_scalar_mul(wl_r[:], wl_tmp[:], gamma_sb[:])

    ids = (identity, identity_r, id1)
    if nreps == 1:
        _emit_body(nc, tc, const, resident, x_sb, x_tiled, out_tiled,
                   ids, ones_b, w_r, wl_r, first=True)
    else:
        # dev-harness timing build: hardware loop re-running the identical
        # body (same inputs/outputs each iteration)
        _emit_body(nc, tc, const, resident, x_sb, x_tiled, out_tiled,
                   ids, ones_b, w_r, wl_r, first=True)
        with tc.For_i(0, nreps - 1, 1):
            _emit_body(nc, tc, const, resident, x_sb, x_tiled, out_tiled,
                       ids, ones_b, w_r, wl_r, first=False)


def _emit_body(nc, tc, const, resident, x_sb, x_tiled, out_tiled,
               ids, ones_b, w_r, wl_r, first):
    identity, identity_r, id1 = ids
    # --- resident tensors ------------------------------------------------
    if not first:
        for s in range(4):
            nc.sync.dma_start(
                out=x_sb[:, s * 8:(s + 1) * 8, :],
                in_=x_tiled[:, s * 8:(s + 1) * 8, :],
            )
    qT_sb = resident.tile([128, L], f32r, tag="qT")          # 16 KB/part
    kT_sb = resident.tile([128, L], f32r, tag="kT")          # 16 KB/part
    v_sb = resident.tile([128, NT, D], bf16, tag="v")        # 8 KB/part

    # --- phase 1: transposes + projections -------------------------------
    # x transposes in f32 (BIR requires f32r matmul inputs to come from a
    # rounding op; DMA-written x can't); v^T transposes in f32r (1.5 c/r,
    # their input comes from an ACT copy that rounds).
    # PSUM->SBUF copies: xt/vt on ACT, q/k/v on DVE (all idle in phase 1).
    # Three-stage software pipeline so the in-order PE never waits on a
    # PSUM->SBUF copy: stage A transposes x(c), stage B projects q/k/v(c-1),
    # stage C back-transposes v(c-2).
    with (
        tc.tile_pool(name="xt", bufs=3) as xtp,
        tc.tile_pool(name="vt", bufs=3) as vtp,
        tc.tile_pool(name="ps_tr", bufs=3, space="PSUM") as ps_tr,
        tc.tile_pool(name="ps_proj", bufs=3, space="PSUM") as ps_proj,
    ):
        def stage_a(c):
            # x^T for this chunk: [128, CK, 512] (C-chunk on dim1), f32r
            xt_c = xtp.tile([128, CK, 512], f32r, name="xt_c")
            for cc in range(CK):
                ps = ps_tr.tile([128, 512], f32, tag="tr", name="ps_t")
                for i in range(4):
                    t = 4 * c + i
                    nc.tensor.transpose(
                        ps[:, i * 128:(i + 1) * 128],
                        x_sb[:, t, cc * 128:(cc + 1) * 128],
                        identity[:],
                    )
                nc.scalar.activation(out=xt_c[:, cc, :], in_=ps[:], func=AF.Copy)
            return xt_c

        def stage_b(c, xt_c):
            cs = slice(c * 512, (c + 1) * 512)
            for name, dstT in (("q", qT_sb), ("k", kT_sb)):
                ps = ps_proj.tile([128, 512], f32, tag="proj", name="ps_p")
                for cc in range(CK):
                    nc.tensor.matmul(
                        ps[:], w_r[name][:, cc, :], xt_c[:, cc, :],
                        start=(cc == 0), stop=(cc == CK - 1),
                    )
                nc.vector.tensor_copy(out=dstT[:, cs], in_=ps[:])
            ps = ps_proj.tile([128, 512], f32, tag="proj", name="ps_p")
            for cc in range(CK):
                nc.tensor.matmul(
                    ps[:], w_r["v"][:, cc, :], xt_c[:, cc, :],
                    start=(cc == 0), stop=(cc == CK - 1),
                )
            vt_c = vtp.tile([128, 512], f32r, name="vt_c")
            nc.scalar.activation(out=vt_c[:], in_=ps[:], func=AF.Copy)
            return vt_c

        def stage_c(c, vt_c):
            ps2 = ps_tr.tile([128, 512], f32r, tag="tr", name="ps_t")
            for i in range(4):
                nc.tensor.transpose(
                    ps2[:, i * 128:(i + 1) * 128],
                    vt_c[:, i * 128:(i + 1) * 128], identity_r[:],
                )
            nc.vector.tensor_copy(out=v_sb[:, 4 * c:4 * c + 4, :], in_=ps2[:])

        stages = []  # (c, xt_c) then (c, vt_c)
        b_in = c_in = None
        for c in range(NCHUNK + 2):
            if c < NCHUNK:
                a_out = stage_a(c)
            if b_in is not None:
                c_next = stage_b(b_in[0], b_in[1])
            if c_in is not None:
                stage_c(c_in[0], c_in[1])
            c_in = (b_in[0], c_next) if b_in is not None else None
            b_in = (c, a_out) if c < NCHUNK else None

    # --- phase 2: attention ----------------------------------------------
    with (
        tc.tile_pool(name="pexp", bufs=3) as pexp,          # [128,4,2,512] bf16 octs
        tc.tile_pool(name="treep", bufs=2) as treep,        # tree levels bf16
        tc.tile_pool(name="asb", bufs=2) as asb,
        tc.tile_pool(name="osb", bufs=2) as osb,
        tc.tile_pool(name="dsb", bufs=2) as dsb,
        tc.tile_pool(name="ps_s", bufs=2, space="PSUM") as ps_s,      # 2x2 banks
        tc.tile_pool(name="ps_acc", bufs=2, space="PSUM") as ps_acc,  # 2 banks
        tc.tile_pool(name="ps_den", bufs=1, space="PSUM") as ps_den,  # 1 bank
        tc.tile_pool(name="ps_po", bufs=1, space="PSUM") as ps_po,    # 1 bank
    ):
        def emit_tail_a(c, acc, den):
            # A~^T to SBUF (f32r) for MM4 (GPSIMD cannot access PSUM -> DVE)
            a_sb = asb.tile([128, 512], f32r, tag="a_sb", name="a_sb")
            nc.vector.tensor_copy(out=a_sb[:], in_=acc[:])
            # denominator row -> free dim of partition 0, then transpose to
            # per-partition scale columns (into the MM4 PSUM bank, whose
            # first 4 columns are free until MM4 m=0 lands after sc_raw)
            tall = dsb.tile([1, 512], f32, tag="tall", name="tall")
            nc.vector.tensor_copy(out=tall[:], in_=den[:])
            po1 = ps_po.tile([128, 2, C], f32, tag="po", name="po1")
            for m in range(4):
                nc.tensor.transpose(
                    po1[:, 0, m:m + 1], tall[0:1, m * 128:(m + 1) * 128], id1[:]
                )
            sc_raw = dsb.tile([128, 4], f32, tag="scraw", name="scraw")
            nc.vector.tensor_copy(out=sc_raw[:], in_=po1[:, 0, 0:4])
            sc = dsb.tile([128, 4], f32, tag="sc", name="sc")
            nc.vector.reciprocal(out=sc[:], in_=sc_raw[:])

            o_sb = osb.tile([128, 4, C], f32, tag="o_sb", name="o_sb")
            for m in range(2):
                t = 4 * c + m
                nc.tensor.matmul(
                    po1[:, m, :], a_sb[:, m * 128:(m + 1) * 128], wl_r[:],
                    start=True, stop=True,
                )
                nc.vector.scalar_tensor_tensor(
                    out=o_sb[:, m, :], in0=po1[:, m, :], scalar=sc[:, m:m + 1],
                    in1=x_sb[:, t, :], op0=ALU.mult, op1=ALU.add,
                )
            return a_sb, sc, o_sb

        def emit_tail_b(c, a_sb, sc, o_sb):
            po2 = ps_po.tile([128, 2, C], f32, tag="po", name="po2")
            for m in range(2, 4):
                t = 4 * c + m
                nc.tensor.matmul(
                    po2[:, m - 2, :], a_sb[:, m * 128:(m + 1) * 128], wl_r[:],
                    start=True, stop=True,
                )
                nc.vector.scalar_tensor_tensor(
                    out=o_sb[:, m, :], in0=po2[:, m - 2, :], scalar=sc[:, m:m + 1],
                    in1=x_sb[:, t, :], op0=ALU.mult, op1=ALU.add,
                )
            nc.sync.dma_start(
                out=out_tiled[:, 4 * c:4 * c + 4, :], in_=o_sb[:]
            )

        pending_tail = None
        tail_mid = None
        for c in range(NCHUNK):
            cs = slice(c * 512, (c + 1) * 512)
            acc = ps_acc.tile([128, 512], f32)
            den = ps_den.tile([1, 512], f32)
            l3buf = treep.tile([128, 4, 512], bf16, tag="l3", name="l3buf")

            def mm2_for(poct, g, acc=acc):
                for j in range(2):
                    lk = 2 * g + j
                    nc.tensor.matmul(
                        acc[:], v_sb[:, lk, :], poct[:, g % 4, j, :],
                        start=(lk == 0), stop=(lk == NT - 1),
                        skip_group_check=True,
                    )

            pipe = []
            mm3_pending = []
            p_oct = None

            def emit_mm3(o, l3buf=l3buf, den=den):
                nc.tensor.matmul(
                    den[:], ones_b[:], l3buf[:, o, :],
                    start=(o == 0), stop=(o == 3),
                    skip_group_check=True,
                )

            for g in range(NG):
                if g % 4 == 0:
                    p_oct = pexp.tile([128, 4, 2, 512], bf16, tag="oct",
                                      name="p_oct")
                s_pair = ps_s.tile([128, 2, 512], f32, tag="s", name="s_pair")
                for j in range(2):
                    lk = 2 * g + j
                    nc.tensor.matmul(
                        s_pair[:, j, :], kT_sb[:, lk * 128:(lk + 1) * 128],
                        qT_sb[:, cs], start=True, stop=True,
                    )
                import os
                if os.environ.get("KEXP", "pair") == "single":
                    for j in range(2):
                        nc.scalar.activation(out=p_oct[:, g % 4, j, :],
                                             in_=s_pair[:, j, :], func=AF.Exp)
                else:
                    nc.scalar.activation(out=p_oct[:, g % 4, :, :], in_=s_pair[:],
                                         func=AF.Exp)
                if os.environ.get("KTREE", "oct") == "flat":
                    # flat variant: contiguous 2-D APs only
                    l1 = treep.tile([128, 4, 512], bf16, tag="l1", name="l1")
                    nc.vector.tensor_tensor(
                        out=l1[:, g % 4, :], in0=p_oct[:, g % 4, 0, :],
                        in1=p_oct[:, g % 4, 1, :], op=ALU.add,
                    )
                    if g % 4 == 3:
                        o = g // 4
                        l2 = treep.tile([128, 2, 512], bf16, tag="l2", name="l2")
                        nc.vector.tensor_tensor(
                            out=l2[:, 0, :], in0=l1[:, 0, :], in1=l1[:, 1, :],
                            op=ALU.add,
                        )
                        nc.vector.tensor_tensor(
                            out=l2[:, 1, :], in0=l1[:, 2, :], in1=l1[:, 3, :],
                            op=ALU.add,
                        )
                        nc.vector.tensor_tensor(
                            out=l3buf[:, o, :], in0=l2[:, 0, :], in1=l2[:, 1, :],
                            op=ALU.add,
                        )
                        mm3_pending.append((o, g))
                elif g % 4 == 3:
                    o = g // 4
                    # oct-batched bf16 tree: L1 both pair-sums of the oct in
                    # one strided op, L2 likewise, L3 into the chunk buffer;
                    # MM3 accumulates the four oct-sums into the denominator
                    # (keeps the chunk-end critical chain short)
                    l1 = treep.tile([128, 4, 512], bf16, tag="l1", name="l1")
                    nc.vector.tensor_tensor(
                        out=l1[:], in0=p_oct[:, :, 0, :], in1=p_oct[:, :, 1, :],
                        op=ALU.add,
                    )
                    l1v = l1[:].rearrange("p (a b) f -> p a b f", b=2)
                    l2 = treep.tile([128, 2, 512], bf16, tag="l2", name="l2")
                    nc.vector.tensor_tensor(
                        out=l2[:], in0=l1v[:, :, 0, :], in1=l1v[:, :, 1, :],
                        op=ALU.add,
                    )
                    nc.vector.tensor_tensor(
                        out=l3buf[:, o, :], in0=l2[:, 0, :], in1=l2[:, 1, :],
                        op=ALU.add,
                    )
                    mm3_pending.append((o, g))
                pipe.append((p_oct, g))
                if len(pipe) > PIPE_DEPTH:
                    mm2_for(*pipe.pop(0))
                # emit each oct's denominator matmul >=2 groups after its
                # tree ops so the in-order PE never waits on the DVE
                if mm3_pending and g - mm3_pending[0][1] >= 2:
                    emit_mm3(mm3_pending.pop(0)[0])
                # previous chunk's tail interleaves with this chunk's matmul
                # stream instead of stalling the PE at the boundary; split in
                # two so the single MM4 PSUM bank never backs up the PE queue
                if g == 2 and pending_tail is not None:
                    tail_mid = (pending_tail[0],) + emit_tail_a(*pending_tail)
                    pending_tail = None
                elif g == 4 and tail_mid is not None:
                    emit_tail_b(*tail_mid)
                    tail_mid = None
            while pipe:
                mm2_for(*pipe.pop(0))
                if mm3_pending:
                    emit_mm3(mm3_pending.pop(0)[0])
            while mm3_pending:
                emit_mm3(mm3_pending.pop(0)[0])
            pending_tail = (c, acc, den)
        tail_mid = (pending_tail[0],) + emit_tail_a(*pending_tail)
        emit_tail_b(*tail_mid)


_NC_CACHE = {}


def _build(nreps=1):
    """Build the Bass module; nreps>1 repeats the whole body (for marginal-
    time measurement in the dev harness — grading path uses nreps=1)."""
    if nreps not in _NC_CACHE:
        from contextlib import ExitStack

        nc = bacc.Bacc("TRN2", target_bir_lowering=False)
        with tile.TileContext(nc) as tc:
            with ExitStack() as ctx:
                _emit(nc, tc, ctx, nreps=nreps)
        nc.compile()
        _NC_CACHE[nreps] = nc
    return _NC_CACHE[nreps]


def kernel(x, Wq, Wk, Wv, Wlast, gamma):
    assert x.shape == (B, H, W, C), x.shape
    nc = _build()
    xf = np.ascontiguousarray(x, dtype=np.float32).reshape(B, L, C)
    in_maps = [
        {
            "x": xf[b],
            "Wq": np.ascontiguousarray(Wq, dtype=np.float32),
            "Wk": np.ascontiguousarray(Wk, dtype=np.float32),
            "Wv": np.ascontiguousarray(Wv, dtype=np.float32),
            "Wlast": np.ascontiguousarray(Wlast, dtype=np.float32),
            "gamma": np.ascontiguousarray(gamma, dtype=np.float32),
        }
        for b in range(B)
    ]
    res = run_bass_kernel_spmd(nc, in_maps, core_ids=list(range(B)))
    out = np.stack([res.results[b]["out"] for b in range(B)], axis=0)
    return out.reshape(B, H, W, C)


# revision 26
# speedup vs baseline: 1.0652x; 1.0652x over previous
"""Trainium2 Bass kernel for nn_AttentionLayer (B=8, H=W=64, C=256, D=128).

Strategy: data-parallel over batch B=8 across the 8 NeuronCores (attention is
independent per batch element). Per core, for its batch element's x [L=4096,
C=256]:

  phase 1: PE-transpose x tiles (f32r, 1.5 c/r) -> xT, project q^T,k^T [D, L]
           (f32r) and v^T, then PE-transpose v^T -> v [L, D] (bf16).
           PSUM->SBUF copies split between ACT and DVE (both idle here).
  phase 2 (per 512-wide Lq chunk, per pair of 128-row Lk tiles,
           software-pipelined):
        MM1 pair: S^T [128,2,512] = k_tile @ q_chunk^T     (PE, fp32r,
                  into a 2-bank PSUM pair tile)
        exp:      ONE activation instr per pair -> bf16 P~^T into an oct
                  buffer [128,4,2,512]                      (ACT)
        MM2:      A~^T += v_tile^T @ P~^T tile              (PE, bf16 moving)
        denominator: bf16 DVE reduction tree, oct-batched (1285/751/485 ns
                  strided tensor_tensor adds), full depth -> ONE
                  MM3 (ones^T @ treetop) per chunk          (PE)
      tail (split in two, deferred into the next chunk's matmul stream):
        denom -> per-partition scale via 4 tiny PE transposes into the MM4
        PSUM bank, reciprocal (gamma folded into Wlast at setup);
        MM4: out = A~ @ Wlast (2+2 matmuls, single shared PSUM bank);
        DVE: out*scale + x; one batched output DMA per chunk.

Engine balance (cost-model): ACT ~133us (128 paired exps - the bottleneck),
PE ~132us, DVE ~105us (tree+tails), Pool ~10us. PSUM: 2x2-bank S pairs +
2 acc + 1 den + 1 MM4/scale = 8 banks.

Numerics: matmuls fp32r (MM1/projections) and bf16 (MM2/MM3 moving side)
with fp32 PSUM accumulation. Softmax skips max-subtraction: logits are
O(+-45) so exp stays inside fp32/bf16 range and softmax is shift-invariant.
P~, v, and the denominator tree are bf16 (~0.2% relative), attention
weights' bf16 error largely cancels after normalization.
"""

import numpy as np

import concourse.bass as bass
import concourse.mybir as mybir
import concourse.tile as tile
from concourse import bacc
from concourse.masks import make_identity
from concourse.bass_utils import run_bass_kernel_spmd

f32 = mybir.dt.float32
f32r = mybir.dt.float32r
bf16 = mybir.dt.bfloat16
AF = mybir.ActivationFunctionType
ALU = mybir.AluOpType

B, H, W, C, D = 8, 64, 64, 256, 128
L = H * W            # 4096
NT = L // 128        # 32 L-tiles of 128 rows
NCHUNK = L // 512    # 8 Lq chunks of 512
CK = C // 128        # 2 C-chunks
NG = NT // 2         # 16 tile-pairs (groups) per chunk
PIPE_DEPTH = 3       # groups in flight before MM2 drains


def _emit(nc, tc, ctx, nreps=1):
    x_d = nc.declare_dram_parameter("x", [L, C], f32, isOutput=False)
    wq_d = nc.declare_dram_parameter("Wq", [C, D], f32, isOutput=False)
    wk_d = nc.declare_dram_parameter("Wk", [C, D], f32, isOutput=False)
    wv_d = nc.declare_dram_parameter("Wv", [C, D], f32, isOutput=False)
    wl_d = nc.declare_dram_parameter("Wlast", [D, C], f32, isOutput=False)
    g_d = nc.declare_dram_parameter("gamma", [1], f32, isOutput=False)
    out_d = nc.declare_dram_parameter("out", [L, C], f32, isOutput=True)

    x_tiled = x_d[:].rearrange("(t p) c -> p t c", p=128)      # [128, NT, C]
    out_tiled = out_d[:].rearrange("(t p) c -> p t c", p=128)  # [128, NT, C]

    const = ctx.enter_context(tc.tile_pool(name="const", bufs=1))
    resident = ctx.enter_context(tc.tile_pool(name="resident", bufs=1))

    # --- constants (weight DMAs first: tiny, needed by the projections) --
    gamma_sb = const.tile([128, 1], f32)
    nc.sync.dma_start(out=gamma_sb[:], in_=g_d[:].to_broadcast((128, 1)))
    w_r = {}
    wtmps = []
    for name, wd in (("q", wq_d), ("k", wk_d), ("v", wv_d)):
        wtmp = const.tile([128, CK, D], f32, name=f"wtmp_{name}")
        nc.sync.dma_start(out=wtmp[:], in_=wd[:].rearrange("(cc p) d -> p cc d", p=128))
        wtmps.append((name, wtmp))
    wl_tmp = const.tile([128, C], f32)
    nc.sync.dma_start(out=wl_tmp[:], in_=wl_d[:])

    # x DMAs right behind (small first slice) so the PE can start
    # transposing ~3us in
    x_sb = resident.tile([128, NT, C], f32, tag="x_sb")      # 32 KB/part
    nc.sync.dma_start(out=x_sb[:, 0:4, :], in_=x_tiled[:, 0:4, :])
    for s in range(3):
        nc.sync.dma_start(
            out=x_sb[:, 4 + s * 8:12 + s * 8, :],
            in_=x_tiled[:, 4 + s * 8:12 + s * 8, :],
        )
    nc.sync.dma_start(out=x_sb[:, 28:32, :], in_=x_tiled[:, 28:32, :])

    identity = const.tile([128, 128], f32)
    make_identity(nc, identity[:])
    identity_r = const.tile([128, 128], f32r)
    nc.vector.tensor_copy(out=identity_r[:], in_=identity[:])
    ones_b = const.tile([128, 1], bf16)
    nc.vector.memset(ones_b[:], 1.0)
    id1 = const.tile([1, 1], f32)
    nc.vector.memset(id1[:], 1.0)
    # weights: lhsT chunks [C128, D] for q/k/v (f32r); [D, C] for last with
    # gamma folded in (f32r)
    for name, wtmp in wtmps:
        wr = const.tile([128, CK, D], f32r, name=f"w_{name}")
        nc.vector.tensor_copy(out=wr[:], in_=wtmp[:])
        w_r[name] = wr
    wl_r = const.tile([128, C], f32r)
    nc.vector.tensor_scalar_mul(wl_r[:], wl_tmp[:], gamma_sb[:])

    ids = (identity, identity_r, id1)
    if nreps == 1:
        _emit_body(nc, tc, const, resident, x_sb, x_tiled, out_tiled,
                   ids, ones_b, w_r, wl_r, first=True)
    else:
        # dev-harness timing build: hardware loop re-running the identical
        # body (same inputs/outputs each iteration)
        _emit_body(nc, tc, const, resident, x_sb, x_tiled, out_tiled,
                   ids, ones_b, w_r, wl_r, first=True)
        with tc.For_i(0, nreps - 1, 1):
            _emit_body(nc, tc, const, resident, x_sb, x_tiled, out_tiled,
                       ids, ones_b, w_r, wl_r, first=False)


def _emit_body(nc, tc, const, resident, x_sb, x_tiled, out_tiled,
               ids, ones_b, w_r, wl_r, first):
    identity, identity_r, id1 = ids
    # --- resident tensors ------------------------------------------------
    if not first:
        for s in range(4):
            nc.sync.dma_start(
                out=x_sb[:, s * 8:(s + 1) * 8, :],
                in_=x_tiled[:, s * 8:(s + 1) * 8, :],
            )
    # q^T/k^T in bf16: rotating f32r stationaries leave the PE waiting on
    # unpipelined 4-byte weight loads (~+450ns per MM1 pair measured);
    # bf16 stationaries prefetch via standalone Ldweights
    qT_sb = resident.tile([128, L], bf16, tag="qT")          # 8 KB/part
    kT_sb = resident.tile([128, L], bf16, tag="kT")          # 8 KB/part
    v_sb = resident.tile([128, NT, D], bf16, tag="v")        # 8 KB/part

    # --- phase 1: transposes + projections -------------------------------
    # x transposes in f32 (BIR requires f32r matmul inputs to come from a
    # rounding op; DMA-written x can't); v^T transposes in f32r (1.5 c/r,
    # their input comes from an ACT copy that rounds).
    # PSUM->SBUF copies: xt/vt on ACT, q/k/v on DVE (all idle in phase 1).
    # Three-stage software pipeline so the in-order PE never waits on a
    # PSUM->SBUF copy: stage A transposes x(c), stage B projects q/k/v(c-1),
    # stage C back-transposes v(c-2).
    with (
        tc.tile_pool(name="xt", bufs=3) as xtp,
        tc.tile_pool(name="vt", bufs=3) as vtp,
        tc.tile_pool(name="ps_tr", bufs=3, space="PSUM") as ps_tr,
        tc.tile_pool(name="ps_proj", bufs=3, space="PSUM") as ps_proj,
    ):
        def stage_a(c):
            # x^T for this chunk: [128, CK, 512] (C-chunk on dim1), f32r
            xt_c = xtp.tile([128, CK, 512], f32r, name="xt_c")
            for cc in range(CK):
                ps = ps_tr.tile([128, 512], f32, tag="tr", name="ps_t")
                for i in range(4):
                    t = 4 * c + i
                    nc.tensor.transpose(
                        ps[:, i * 128:(i + 1) * 128],
                        x_sb[:, t, cc * 128:(cc + 1) * 128],
                        identity[:],
                    )
                nc.scalar.activation(out=xt_c[:, cc, :], in_=ps[:], func=AF.Copy)
            return xt_c

        def stage_b(c, xt_c):
            cs = slice(c * 512, (c + 1) * 512)
            for name, dstT in (("q", qT_sb), ("k", kT_sb)):
                ps = ps_proj.tile([128, 512], f32, tag="proj", name="ps_p")
                for cc in range(CK):
                    nc.tensor.matmul(
                        ps[:], w_r[name][:, cc, :], xt_c[:, cc, :],
                        start=(cc == 0), stop=(cc == CK - 1),
                    )
                nc.vector.tensor_copy(out=dstT[:, cs], in_=ps[:])
            ps = ps_proj.tile([128, 512], f32, tag="proj", name="ps_p")
            for cc in range(CK):
                nc.tensor.matmul(
                    ps[:], w_r["v"][:, cc, :], xt_c[:, cc, :],
                    start=(cc == 0), stop=(cc == CK - 1),
                )
            vt_c = vtp.tile([128, 512], f32r, name="vt_c")
            nc.scalar.activation(out=vt_c[:], in_=ps[:], func=AF.Copy)
            return vt_c

        def stage_c(c, vt_c):
            ps2 = ps_tr.tile([128, 512], f32r, tag="tr", name="ps_t")
            for i in range(4):
                nc.tensor.transpose(
                    ps2[:, i * 128:(i + 1) * 128],
                    vt_c[:, i * 128:(i + 1) * 128], identity_r[:],
                )
            nc.vector.tensor_copy(out=v_sb[:, 4 * c:4 * c + 4, :], in_=ps2[:])

        stages = []  # (c, xt_c) then (c, vt_c)
        b_in = c_in = None
        for c in range(NCHUNK + 2):
            if c < NCHUNK:
                a_out = stage_a(c)
            if b_in is not None:
                c_next = stage_b(b_in[0], b_in[1])
            if c_in is not None:
                stage_c(c_in[0], c_in[1])
            c_in = (b_in[0], c_next) if b_in is not None else None
            b_in = (c, a_out) if c < NCHUNK else None

    # --- phase 2: attention ----------------------------------------------
    # PSUM: 3x2-bank S pairs + 1 acc + 1 shared den/MM4 = 8 banks
    with (
        tc.tile_pool(name="pexp", bufs=3) as pexp,          # [128,4,2,512] bf16 octs
        tc.tile_pool(name="treep", bufs=2) as treep,        # tree levels bf16
        tc.tile_pool(name="asb", bufs=2) as asb,
        tc.tile_pool(name="osb", bufs=2) as osb,
        tc.tile_pool(name="dsb", bufs=2) as dsb,
        tc.tile_pool(name="ps_s", bufs=3, space="PSUM") as ps_s,      # 3x2 banks
        tc.tile_pool(name="ps_acc", bufs=1, space="PSUM") as ps_acc,  # 1 bank
        tc.tile_pool(name="ps_dp", bufs=1, space="PSUM") as ps_dp,    # 1 bank
    ):
        def emit_tail_a(c, acc, den):
            # A~^T to SBUF (f32r) for MM4 (GPSIMD cannot access PSUM -> DVE)
            a_sb = asb.tile([128, 512], f32r, tag="a_sb", name="a_sb")
            nc.vector.tensor_copy(out=a_sb[:], in_=acc[:])
            # denominator row -> free dim of partition 0, then transpose to
            # per-partition scale columns (into the MM4 PSUM bank, whose
            # first 4 columns are free until MM4 m=0 lands after sc_raw)
            tall = dsb.tile([1, 512], f32, tag="tall", name="tall")
            nc.vector.tensor_copy(out=tall[:], in_=den[:])
            po1 = ps_dp.tile([128, 2, C], f32, tag="t", name="po1")
            for m in range(4):
                nc.tensor.transpose(
                    po1[:, 0, m:m + 1], tall[0:1, m * 128:(m + 1) * 128], id1[:]
                )
            sc_raw = dsb.tile([128, 4], f32, tag="scraw", name="scraw")
            nc.vector.tensor_copy(out=sc_raw[:], in_=po1[:, 0, 0:4])
            sc = dsb.tile([128, 4], f32, tag="sc", name="sc")
            nc.vector.reciprocal(out=sc[:], in_=sc_raw[:])

            o_sb = osb.tile([128, 4, C], f32, tag="o_sb", name="o_sb")
            for m in range(2):
                t = 4 * c + m
                nc.tensor.matmul(
                    po1[:, m, :], a_sb[:, m * 128:(m + 1) * 128], wl_r[:],
                    start=True, stop=True,
                )
                nc.vector.scalar_tensor_tensor(
                    out=o_sb[:, m, :], in0=po1[:, m, :], scalar=sc[:, m:m + 1],
                    in1=x_sb[:, t, :], op0=ALU.mult, op1=ALU.add,
                )
            return a_sb, sc, o_sb

        def emit_tail_b(c, a_sb, sc, o_sb):
            po2 = ps_dp.tile([128, 2, C], f32, tag="t", name="po2")
            for m in range(2, 4):
                t = 4 * c + m
                nc.tensor.matmul(
                    po2[:, m - 2, :], a_sb[:, m * 128:(m + 1) * 128], wl_r[:],
                    start=True, stop=True,
                )
                nc.vector.scalar_tensor_tensor(
                    out=o_sb[:, m, :], in0=po2[:, m - 2, :], scalar=sc[:, m:m + 1],
                    in1=x_sb[:, t, :], op0=ALU.mult, op1=ALU.add,
                )
            nc.sync.dma_start(
                out=out_tiled[:, 4 * c:4 * c + 4, :], in_=o_sb[:]
            )

        pending_top = None   # (c, acc, top) awaiting MM3 + tail
        pending_tail = None  # (c, acc, den) after MM3
        tail_mid = None
        for c in range(NCHUNK):
            cs = slice(c * 512, (c + 1) * 512)
            acc = ps_acc.tile([128, 512], f32)
            l3buf = treep.tile([128, 4, 512], bf16, tag="l3", name="l3buf")

            # previous chunk's denominator matmul: the tree top finished
            # during that chunk's MM2 drain, so the PE never waits here
            if pending_top is not None:
                pc, pacc, ptop = pending_top
                den = ps_dp.tile([1, 512], f32, tag="t", name="den")
                nc.tensor.matmul(den[:], ones_b[:], ptop[:],
                                 start=True, stop=True, skip_group_check=True)
                pending_tail = (pc, pacc, den)
                pending_top = None

            def mm2_for(poct, g, acc=acc):
                for j in range(2):
                    lk = 2 * g + j
                    nc.tensor.matmul(
                        acc[:], v_sb[:, lk, :], poct[:, g % 4, j, :],
                        start=(lk == 0), stop=(lk == NT - 1),
                        skip_group_check=True,
                    )

            pipe = []
            p_oct = None
            for g in range(NG):
                if g % 4 == 0:
                    p_oct = pexp.tile([128, 4, 2, 512], bf16, tag="oct",
                                      name="p_oct")
                s_pair = ps_s.tile([128, 2, 512], f32, tag="s", name="s_pair")
                for j in range(2):
                    lk = 2 * g + j
                    nc.tensor.matmul(
                        s_pair[:, j, :], kT_sb[:, lk * 128:(lk + 1) * 128],
                        qT_sb[:, cs], start=True, stop=True,
                    )
                nc.scalar.activation(out=p_oct[:, g % 4, :, :], in_=s_pair[:],
                                     func=AF.Exp)
                if g % 4 == 3:
                    o = g // 4
                    # oct-batched bf16 tree: L1 both pair-sums of the oct in
                    # one strided op, L2 likewise, L3 into the chunk buffer
                    l1 = treep.tile([128, 4, 512], bf16, tag="l1", name="l1")
                    nc.vector.tensor_tensor(
                        out=l1[:], in0=p_oct[:, :, 0, :], in1=p_oct[:, :, 1, :],
                        op=ALU.add,
                    )
                    l1v = l1[:].rearrange("p (a b) f -> p a b f", b=2)
                    l2 = treep.tile([128, 2, 512], bf16, tag="l2", name="l2")
                    nc.vector.tensor_tensor(
                        out=l2[:], in0=l1v[:, :, 0, :], in1=l1v[:, :, 1, :],
                        op=ALU.add,
                    )
                    nc.vector.tensor_tensor(
                        out=l3buf[:, o, :], in0=l2[:, 0, :], in1=l2[:, 1, :],
                        op=ALU.add,
                    )
                    if o == 3:
                        l3v = l3buf[:].rearrange("p (a b) f -> p a b f", b=2)
                        l4 = treep.tile([128, 2, 512], bf16, tag="l4", name="l4")
                        nc.vector.tensor_tensor(
                            out=l4[:], in0=l3v[:, :, 0, :], in1=l3v[:, :, 1, :],
                            op=ALU.add,
                        )
                        top = treep.tile([128, 512], bf16, tag="top", name="top")
                        nc.vector.tensor_tensor(
                            out=top[:], in0=l4[:, 0, :], in1=l4[:, 1, :],
                            op=ALU.add,
                        )
                pipe.append((p_oct, g))
                if len(pipe) > PIPE_DEPTH:
                    mm2_for(*pipe.pop(0))
                # previous chunk's tail interleaves with this chunk's matmul
                # stream instead of stalling the PE at the boundary; split in
                # two so the single MM4 PSUM bank never backs up the PE queue
                if g == 2 and pending_tail is not None:
                    tail_mid = (pending_tail[0],) + emit_tail_a(*pending_tail)
                    pending_tail = None
                elif g == 4 and tail_mid is not None:
                    emit_tail_b(*tail_mid)
                    tail_mid = None
            while pipe:
                mm2_for(*pipe.pop(0))
            pending_top = (c, acc, top)
        # final chunk: MM3 + both tail halves after the drain
        pc, pacc, ptop = pending_top
        den = ps_dp.tile([1, 512], f32, tag="t", name="den")
        nc.tensor.matmul(den[:], ones_b[:], ptop[:],
                         start=True, stop=True, skip_group_check=True)
        tail_mid = (pc,) + emit_tail_a(pc, pacc, den)
        emit_tail_b(*tail_mid)


_NC_CACHE = {}


def _build(nreps=1):
    """Build the Bass module; nreps>1 repeats the whole body (for marginal-
    time measurement in the dev harness — grading path uses nreps=1)."""
    if nreps not in _NC_CACHE:
        from contextlib import ExitStack

        nc = bacc.Bacc("TRN2", target_bir_lowering=False)
        with tile.TileContext(nc) as tc:
            with ExitStack() as ctx:
                _emit(nc, tc, ctx, nreps=nreps)
        nc.compile()
        _NC_CACHE[nreps] = nc
    return _NC_CACHE[nreps]


def kernel(x, Wq, Wk, Wv, Wlast, gamma):
    assert x.shape == (B, H, W, C), x.shape
    nc = _build()
    xf = np.ascontiguousarray(x, dtype=np.float32).reshape(B, L, C)
    in_maps = [
        {
            "x": xf[b],
            "Wq": np.ascontiguousarray(Wq, dtype=np.float32),
            "Wk": np.ascontiguousarray(Wk, dtype=np.float32),
            "Wv": np.ascontiguousarray(Wv, dtype=np.float32),
            "Wlast": np.ascontiguousarray(Wlast, dtype=np.float32),
            "gamma": np.ascontiguousarray(gamma, dtype=np.float32),
        }
        for b in range(B)
    ]
    res = run_bass_kernel_spmd(nc, in_maps, core_ids=list(range(B)))
    out = np.stack([res.results[b]["out"] for b in range(B)], axis=0)
    return out.reshape(B, H, W, C)
